# revision 35
# baseline (speedup 1.0000x reference)
# Multi-head attention (B=2, S=2048, D=1024, H=16) on 8 TRN2 NeuronCores.
#
# Sharding (hardcoded): core c in [0..8) handles batch b = c//4 and head
# group g = c%4 (4 heads = 256 output features of wq/wk/wv, 256 input rows
# of wo). Each core computes a partial output projection [S, D]; the host
# sums the 4 partials per batch and adds wo_bias (row-parallel unshard).
#
# Design (engines balanced around the Scalar exp floor, ~147us/core):
#   - all matmuls bf16 (fp8 fails the 2e-2 tolerance: random-sign dots keep
#     the per-element quant error, and scores enter exp).
#   - activations enter transposed ([D, S]); scores computed transposed
#     (S^T[k, q]) so softmax(P) feeds P@V directly; denominator comes free
#     as a ones-column appended to each head's V block.
#   - output projection contracts K=128 by packing head PAIRS into
#     otn2[128, t, S]; odd heads reach partitions 64..127 via a small
#     SBUF->SBUF DMA (engines cannot shift partitions).
#   - uniform deferred-by-one pipeline: each head's 16 score-kt "slots"
#     carry the PREVIOUS head's PV (2 kt-pairs/slot over slots 0-7) plus
#     V-proj tiles / K-t1 / Q-qc1 chunks / outproj tiles as PE fillers, so
#     exp sees a steady kt stream from ~12us in.
#   - softmax normalization split: reciprocal + PSUM-freeing copy + DRAM
#     bounce broadcast at slot 8; the consuming multiply at slot 15 so the
#     bounce latency never blocks the Vector stream head-of-line.
#   - last head: PV inlined (slots 8-15), reciprocal broadcast via a tiny
#     PE matmul (no DRAM bounce on the critical tail), outproj qc1 drains
#     on Scalar (exp is done by then).
import functools
import sys

import numpy as np

try:
    import concourse  # noqa: F401
except ImportError:  # harness env without the default path
    sys.path.insert(0, "/opt/trn_rl_repo")
    sys.path.insert(0, "/opt/pypackages")

import ml_dtypes

BF16 = ml_dtypes.bfloat16

B, S, D, H = 2, 2048, 1024, 16
HD = D // H          # 64
NCORES = 8
GH = 4               # head groups (tensor-parallel)
HPG = H // GH        # heads per group = 4
DG = D // GH         # features per group = 256
P = 128              # partitions
TDIN = D // P        # 8 din tiles
QC = 2               # q-chunks of 1024 for attention
QW = S // QC         # 1024
KT = S // P          # 16 k tiles
NT2 = DG // P        # 2 dout tiles per group


def build_graph():
    """Build the SPMD Bass graph (identical on all 8 cores)."""
    from contextlib import ExitStack

    from concourse import bacc, mybir, tile

    f32 = mybir.dt.float32
    bf16 = mybir.dt.bfloat16
    EXP = mybir.ActivationFunctionType.Exp
    IDENT = mybir.ActivationFunctionType.Identity

    nc = bacc.Bacc(
        "TRN2", target_bir_lowering=False, debug=False, num_devices=NCORES
    )

    xq = nc.dram_tensor("xq_t", (P, TDIN, S), bf16, kind="ExternalInput")
    xk = nc.dram_tensor("xk_t", (P, TDIN, S), bf16, kind="ExternalInput")
    xv = nc.dram_tensor("xv_t", (P, TDIN, S), bf16, kind="ExternalInput")
    mk = nc.dram_tensor("mask_t", (S, S), bf16, kind="ExternalInput")
    wq = nc.dram_tensor("wq", (P, TDIN, DG), bf16, kind="ExternalInput")
    wk = nc.dram_tensor("wk", (P, TDIN, DG), bf16, kind="ExternalInput")
    wv = nc.dram_tensor("wv", (P, TDIN, DG), bf16, kind="ExternalInput")
    # wo pre-arranged host-side to [128, NT2, D]: partition p = (h%2)*64+hd,
    # tile t = h//2 (head pair), so outproj contracts K=128 over 2 heads.
    wo = nc.dram_tensor("wo", (P, NT2, D), bf16, kind="ExternalInput")
    # q/k biases as per-partition columns [128, NT2]; v bias as a row.
    qb = nc.dram_tensor("qb", (P, NT2), f32, kind="ExternalInput")
    kb = nc.dram_tensor("kb", (P, NT2), f32, kind="ExternalInput")
    vb = nc.dram_tensor("vb", (1, DG), bf16, kind="ExternalInput")
    out = nc.dram_tensor("out", (S, D), bf16, kind="ExternalOutput")

    with tile.TileContext(nc) as tc, ExitStack() as ctx:
        wpool = ctx.enter_context(tc.tile_pool(name="wpool", bufs=1))
        cpool = ctx.enter_context(tc.tile_pool(name="cpool", bufs=1))
        qkpool = ctx.enter_context(tc.tile_pool(name="qk", bufs=1))
        vpool = ctx.enter_context(tc.tile_pool(name="vsb", bufs=1))
        mpool = ctx.enter_context(tc.tile_pool(name="msk", bufs=1))
        opool = ctx.enter_context(tc.tile_pool(name="otn", bufs=1))
        xstage = ctx.enter_context(tc.tile_pool(name="xin", bufs=2))
        ptpool = ctx.enter_context(tc.tile_pool(name="pt", bufs=16))
        npool = ctx.enter_context(tc.tile_pool(name="nrm", bufs=1))
        ospool = ctx.enter_context(tc.tile_pool(name="osbp", bufs=1))
        dpool = ctx.enter_context(tc.tile_pool(name="dscr", bufs=2, space="DRAM"))
        # PSUM: scores [128,1024] x2 (4 banks) + o_ps [65,1024] x1 (2 banks)
        # + proj/outproj [128,512] x2 (2 banks) = 8 banks exactly.
        spspool = ctx.enter_context(tc.tile_pool(name="sps", bufs=2, space="PSUM"))
        opspool = ctx.enter_context(tc.tile_pool(name="ops", bufs=1, space="PSUM"))
        pjpool = ctx.enter_context(tc.tile_pool(name="pjps", bufs=2, space="PSUM"))

        # ---- staging + persistent SBUF tensors ---------------------------
        def stage_x(xdr, c, tag, name, pieces=4):
            xt = xstage.tile([P, TDIN, 512], bf16, tag=tag, name=name)
            w = TDIN // pieces
            for th_ in range(pieces):
                nc.sync.dma_start(
                    xt[:, th_ * w : (th_ + 1) * w, :],
                    xdr.ap()[:, th_ * w : (th_ + 1) * w, c * 512 : (c + 1) * 512],
                )
            return xt

        xq_c = [stage_x(xq, 0, "xq", "xq_c0", pieces=8)]
        wq_sb = wpool.tile([P, TDIN, DG], bf16)
        wk_sb = wpool.tile([P, TDIN, DG], bf16)
        for th_ in range(4):
            nc.sync.dma_start(
                wq_sb[:, th_ * 2 : (th_ + 1) * 2, :],
                wq.ap()[:, th_ * 2 : (th_ + 1) * 2, :],
            )
        xq_c.append(stage_x(xq, 1, "xq", "xq_c1", pieces=8))
        for th_ in range(4):
            nc.sync.dma_start(
                wk_sb[:, th_ * 2 : (th_ + 1) * 2, :],
                wk.ap()[:, th_ * 2 : (th_ + 1) * 2, :],
            )
        xk_c = [stage_x(xk, 0, "xk", "xk_c0", pieces=8)]
        qb_sb = cpool.tile([P, NT2], f32)
        kb_sb = cpool.tile([P, NT2], f32)
        nc.sync.dma_start(qb_sb[:], qb.ap())
        nc.sync.dma_start(kb_sb[:], kb.ap())
        xk_c.append(stage_x(xk, 1, "xk", "xk_c1", pieces=8))

        # mask per-(kt, qc-half): only the qc0 halves are needed during
        # the first four heads, halving early DMA pressure.
        mask_sb = mpool.tile([P, KT, S], bf16)
        mk_r = mk.ap().rearrange("(t p) q -> p t q", p=P)

        def mask_dma(kt, qc):
            return lambda: nc.sync.dma_start(
                mask_sb[:, kt, qc * QW : (qc + 1) * QW],
                mk_r[:, kt, qc * QW : (qc + 1) * QW],
            )

        mask_dma(0, 0)()
        xv_pre = stage_x(xv, 0, "xv", "xv_c0")
        mask_dma(1, 0)()
        wv_sb = wpool.tile([P, TDIN, DG], bf16)
        for th_ in range(2):
            nc.sync.dma_start(
                wv_sb[:, th_ * 4 : (th_ + 1) * 4, :],
                wv.ap()[:, th_ * 4 : (th_ + 1) * 4, :],
            )
        wo_sb = wpool.tile([P, NT2, D], bf16)

        def wo_dma():
            return lambda: nc.sync.dma_start(wo_sb[:], wo.ap())

        vb_sb = cpool.tile([1, DG], bf16)
        nc.sync.dma_start(vb_sb[:], vb.ap())
        ones2 = cpool.tile([1, P], bf16)
        nc.vector.memset(ones2[:], 1.0)
        ones65 = cpool.tile([HD + 1, HD], f32)
        nc.vector.memset(ones65[:], 1.0)

        qT_sb = qkpool.tile([P, NT2, S], bf16)   # q projection, transposed
        kT_sb = qkpool.tile([P, NT2, S], bf16)
        # v blocks: per k-tile, per head: [v(64) | ones] -> 65 cols
        v_sb = vpool.tile([P, KT, HPG * (HD + 1)], bf16)
        nc.vector.memset(
            v_sb[:].rearrange("p s (h x) -> p s h x", h=HPG)[:, :, :, HD : HD + 1],
            1.0,
        )
        # packed normalized attention output: partition (h%2)*64+hd, tile h//2
        otn2 = opool.tile([P, NT2, S], bf16)

        # ---- projection helpers ------------------------------------------
        def emit_qk_chunk(xsb, xoff, wsb, bias_sb, dest, dt, s0, drain_eng):
            """Project one [128 dout, 512 s] tile: 8 acc matmuls + drain."""
            ps = pjpool.tile([P, 512], f32, tag="pj", name=f"pj_{dt}_{s0}_{drain_eng}")
            for ktl in range(TDIN):
                nc.tensor.matmul(
                    ps[:],
                    lhsT=wsb[:, ktl, dt * P : (dt + 1) * P],
                    rhs=xsb[:, ktl, s0 - xoff : s0 - xoff + 512],
                    start=(ktl == 0),
                    stop=(ktl == TDIN - 1),
                )
            if drain_eng == "scalar":
                nc.scalar.activation(
                    dest[:, dt, s0 : s0 + 512], ps[:], IDENT,
                    bias=bias_sb[:, dt : dt + 1], scale=1.0,
                )
            else:
                nc.vector.tensor_scalar_add(
                    dest[:, dt, s0 : s0 + 512], ps[:], bias_sb[:, dt : dt + 1]
                )

        def emit_v_mtile(xsb, xoff, st):
            """V projection for s-tile st ([128 s, 256 dout] + bias + ones)."""
            ps = pjpool.tile([P, 512], f32, tag="pj", name=f"pv_{st}")
            for ktl in range(TDIN):
                nc.tensor.matmul(
                    ps[:, 0:DG],
                    lhsT=xsb[:, ktl, st * P - xoff : (st + 1) * P - xoff],
                    rhs=wv_sb[:, ktl, :],
                    start=(ktl == 0),
                    stop=False,
                )
            nc.tensor.matmul(
                ps[:, 0:DG],
                lhsT=ones2[0:1, :],
                rhs=vb_sb[:],
                start=False,
                stop=True,
            )
            dst = v_sb[:, st, :].rearrange("p (h x) -> p h x", h=HPG)[:, :, 0:HD]
            src = ps[:, 0:DG].rearrange("p (h x) -> p h x", h=HPG)
            nc.vector.tensor_copy(dst, src)

        # ---- attention helpers -------------------------------------------
        def emit_scores_kt(qc, h, kt, gp_ok=True):
            """scores (2 matmuls) -> exp -> mask-mult; returns P tile."""
            t, po = h // 2, (h % 2) * HD
            s_ps = spspool.tile([P, QW], f32, tag="sps", name=f"sps_{qc}_{h}_{kt}")
            for hf in range(2):
                nc.tensor.matmul(
                    s_ps[:, hf * 512 : (hf + 1) * 512],
                    lhsT=kT_sb[po : po + HD, t, kt * P : (kt + 1) * P],
                    rhs=qT_sb[
                        po : po + HD, t,
                        qc * QW + hf * 512 : qc * QW + (hf + 1) * 512,
                    ],
                    start=True,
                    stop=True,
                )
            pt = ptpool.tile([P, QW], bf16, tag="p", name=f"pt_{qc}_{h}_{kt}")
            nc.scalar.activation(pt[:], s_ps[:], EXP, scale=0.125)
            meng = nc.gpsimd if (gp_ok and kt in (4, 9, 14)) else nc.vector
            meng.tensor_mul(
                pt[:], pt[:], mask_sb[:, kt, qc * QW : (qc + 1) * QW]
            )
            return pt

        def emit_pv_kt(h, kt, pt, o_ps):
            for hf in range(2):
                nc.tensor.matmul(
                    o_ps[:, hf * 512 : (hf + 1) * 512],
                    lhsT=v_sb[:, kt, h * 65 : (h + 1) * 65],
                    rhs=pt[:, hf * 512 : (hf + 1) * 512],
                    start=(kt == 0),
                    stop=(kt == KT - 1),
                )

        def emit_norm_a(qc, h, o_ps):
            """Reciprocal + PSUM-freeing copy + DRAM-bounce broadcast DMAs.
            The consuming multiply runs at slot 15, hiding bounce latency."""
            rec65 = npool.tile([HD + 1, QW], f32, tag="rec")
            nc.vector.reciprocal_approx_fast(out=rec65[:], in_=o_ps[:])
            osb = ospool.tile([HD + 1, QW], bf16, tag="osb", name=f"osb_{qc}_{h}")
            nc.vector.tensor_copy(osb[:], o_ps[:])
            scr = dpool.tile([1, QW], f32, tag="scr", name=f"scr_{qc}_{h}")
            nc.sync.dma_start(scr[:], rec65[HD : HD + 1, :])
            rb = npool.tile([HD, QW], f32, tag="rb")
            nc.sync.dma_start(rb[:], scr[:].to_broadcast((HD, QW)))
            return osb, rb

        def emit_norm_b(qc, h, osb, rb):
            """softmax normalization; writes otn2 (odd heads via DMA)."""
            t = h // 2
            if h % 2 == 0:
                nc.vector.tensor_mul(
                    otn2[0:HD, t, qc * QW : (qc + 1) * QW], osb[0:HD, :], rb[:]
                )
            else:
                nc.vector.tensor_mul(osb[0:HD, :], osb[0:HD, :], rb[:])
                nc.sync.dma_start(
                    otn2[HD:P, t, qc * QW : (qc + 1) * QW], osb[0:HD, :]
                )

        def emit_outproj(st, drain="vector"):
            osb2 = ospool.tile([P, D], bf16, tag="outsb", name=f"outsb_{st}",
                               bufs=2)
            for nch in range(2):
                op_ps = pjpool.tile(
                    [P, 512], f32, tag="pj", name=f"ops2_{st}_{nch}"
                )
                for t in range(NT2):
                    nc.tensor.matmul(
                        op_ps[:],
                        lhsT=otn2[:, t, st * P : (st + 1) * P],
                        rhs=wo_sb[:, t, nch * 512 : (nch + 1) * 512],
                        start=(t == 0),
                        stop=(t == NT2 - 1),
                    )
                dst = osb2[:, nch * 512 : (nch + 1) * 512]
                if drain == "scalar":
                    nc.scalar.copy(dst, op_ps[:])
                else:
                    nc.vector.tensor_copy(dst, op_ps[:])
            if drain == "scalar":
                # tail sts: halve the 256KB single-queue transfer (~11us)
                for p in range(2):
                    nc.sync.dma_start(
                        out.ap()[st * P : (st + 1) * P, p * 512 : (p + 1) * 512],
                        osb2[:, p * 512 : (p + 1) * 512],
                    )
            else:
                nc.sync.dma_start(out.ap()[st * P : (st + 1) * P, :], osb2[:])

        # ---- emission schedule -------------------------------------------
        seq = [(qc, h) for qc in range(QC) for h in range(HPG)]

        # prefix: all Q-qc0 + K-t0 chunk 0 only (~4MB of critical DMA);
        # K-t0 c1..c3 stream as n0 fillers just ahead of their score kts.
        for dt in range(NT2):
            for c in range(2):
                emit_qk_chunk(xq_c[c], c * 512, wq_sb, qb_sb, qT_sb, dt,
                              c * 512, "scalar")
        emit_qk_chunk(xk_c[0], 0, wk_sb, kb_sb, kT_sb, 0, 0, "scalar")

        def k0_chunk(c):
            return lambda: emit_qk_chunk(
                xk_c[c], c * 512, wk_sb, kb_sb, kT_sb, 0, c * 512, "vector")

        xv_c = [xv_pre]

        def v_tile(st):
            return lambda: emit_v_mtile(xv_c[st // 4], (st // 4) * 512, st)

        def stage_thunk(lst, xdr, c, tag, name):
            return lambda: lst.append(stage_x(xdr, c, tag, name))

        def k1_chunk(c):
            return lambda: emit_qk_chunk(
                xk_c[4 + c], c * 512, wk_sb, kb_sb, kT_sb, 1, c * 512,
                "vector")

        def q1_chunk(dt, c):
            return lambda: emit_qk_chunk(
                xq_c[c], c * 512, wq_sb, qb_sb, qT_sb, dt, c * 512,
                "vector")

        # filler map: fillers[n][slot] = list of thunks
        fillers = [dict() for _ in range(8)]

        def add(n, slot, thunk):
            fillers[n].setdefault(slot, []).append(thunk)

        # n0: K-t0 c1..c3 just ahead of their score kts; V tiles 0..7;
        # mask qc0-halves staggered; staging DMAs need-ordered.
        add(0, 0, stage_thunk(xk_c, xk, 2, "xk", "xk_c2"))
        add(0, 1, k0_chunk(1))
        add(0, 1, stage_thunk(xv_c, xv, 1, "xv", "xv_c1"))
        add(0, 4, stage_thunk(xk_c, xk, 3, "xk", "xk_c3"))
        add(0, 6, k0_chunk(2))
        add(0, 10, k0_chunk(3))
        for i, st in enumerate(range(8)):
            add(0, [2, 3, 5, 7, 9, 11, 12, 13][i], v_tile(st))
        for kt in range(2, KT):
            add(0, max(0, kt - 4), mask_dma(kt, 0))
        add(0, 8, stage_thunk(xv_c, xv, 2, "xv", "xv_c2"))
        add(0, 11, stage_thunk(xv_c, xv, 3, "xv", "xv_c3"))
        # n1: V tiles 8..15 (PV-n0-kt j runs at slot j, so V-kt by slot kt);
        # xk re-staging for K-t1; wo.
        for st in range(8, 16):
            add(1, st - 8, v_tile(st))
        add(1, 8, stage_thunk(xk_c, xk, 0, "xk", "xk_c0b"))
        add(1, 10, stage_thunk(xk_c, xk, 1, "xk", "xk_c1b"))
        add(1, 12, stage_thunk(xk_c, xk, 2, "xk", "xk_c2b"))
        add(1, 14, stage_thunk(xk_c, xk, 3, "xk", "xk_c3b"))
        add(1, 15, wo_dma())
        # n2: all K-t1 chunks; mask qc1-halves 0..7 staggered
        add(2, 0, k1_chunk(0))
        add(2, 3, k1_chunk(1))
        add(2, 6, k1_chunk(2))
        add(2, 9, k1_chunk(3))
        for kt in range(8):
            add(2, 2 * kt, mask_dma(kt, 1))
        # n3: all Q-qc1 chunks; mask qc1-halves 8..15
        add(3, 0, stage_thunk(xq_c, xq, 2, "xq", "xq_c2"))
        add(3, 2, stage_thunk(xq_c, xq, 3, "xq", "xq_c3"))
        add(3, 4, q1_chunk(0, 2))
        add(3, 7, q1_chunk(0, 3))
        add(3, 10, q1_chunk(1, 2))
        add(3, 13, q1_chunk(1, 3))
        for kt in range(8, KT):
            add(3, 2 * (kt - 8), mask_dma(kt, 1))
        # n5/n6: outproj qc0 tiles (norm-B of n3 lands at n5 slot 7)
        for i, st in enumerate(range(0, 4)):
            add(5, 9 + 2 * i, lambda st=st: emit_outproj(st))
        for i, st in enumerate(range(4, 8)):
            add(6, 9 + 2 * i, lambda st=st: emit_outproj(st))

        last = len(seq) - 1
        heads = []  # (qc, h, pts, o_ps) per head-seq index
        pnorm = None
        for n, (qc, h) in enumerate(seq):
            o_ps = opspool.tile([HD + 1, QW], f32, tag="ops",
                                name=f"ops_{qc}_{h}")
            pts = []
            for kt in range(KT):
                # norm of head n-2: phase A at slot 0 (frees its PSUM gen
                # right before this window's PV starts), phase B at slot 7.
                if n >= 2 and kt == 0:
                    pqc2, ph2, ppts2, po_ps2 = heads[n - 2]
                    pnorm = (pqc2, ph2) + emit_norm_a(pqc2, ph2, po_ps2)
                for thunk in fillers[n].get(kt, ()):
                    thunk()
                if n >= 1:
                    pqc, ph, ppts, po_ps = heads[n - 1]
                    if n == last:
                        if kt < 8:
                            for j in (2 * kt, 2 * kt + 1):
                                emit_pv_kt(ph, j, ppts[j], po_ps)
                    else:
                        emit_pv_kt(ph, kt, ppts[kt], po_ps)
                pts.append(emit_scores_kt(qc, h, kt, gp_ok=(n != last)))
                if n == last and kt == 8:
                    pqc, ph, ppts, po_ps = heads[n - 1]
                    pn6 = (pqc, ph) + emit_norm_a(pqc, ph, po_ps)
                if n == last and kt >= 8:
                    for j in (2 * (kt - 8), 2 * (kt - 8) + 1):
                        emit_pv_kt(h, j, pts[j], o_ps)
                if pnorm is not None and kt == 7:
                    emit_norm_b(*pnorm)
                    pnorm = None
                if n == last and kt == 15:
                    emit_norm_b(*pn6)
            heads.append((qc, h, pts, o_ps))
        prev = heads[-1]

        # tail: last head's norm via a PE broadcast of the reciprocal row
        # (no DRAM bounce), then outproj qc1 with Scalar drains.
        pqc, ph, ppts, po_ps = prev
        rec65 = npool.tile([HD + 1, QW], f32, tag="rec")
        nc.vector.reciprocal_approx_fast(out=rec65[:], in_=po_ps[:])
        osb = ospool.tile([HD + 1, QW], bf16, tag="osb", name="osb_last")
        nc.vector.tensor_copy(osb[:], po_ps[:])
        t = ph // 2
        for hf in range(2):
            rb_ps = pjpool.tile([HD, 512], f32, tag="pj", name=f"rbps_{hf}")
            nc.tensor.matmul(
                rb_ps[:],
                lhsT=ones65[HD : HD + 1, 0:HD],
                rhs=rec65[HD : HD + 1, hf * 512 : (hf + 1) * 512],
                start=True,
                stop=True,
            )
            nc.vector.tensor_mul(
                osb[0:HD, hf * 512 : (hf + 1) * 512],
                osb[0:HD, hf * 512 : (hf + 1) * 512],
                rb_ps[:],
            )
        nc.sync.dma_start(
            otn2[HD:P, t, pqc * QW : (pqc + 1) * QW], osb[0:HD, :]
        )
        for st in range(8, 16):
            emit_outproj(st, drain="scalar")

    nc.compile()
    return nc


@functools.lru_cache(maxsize=1)
def _graph():
    return build_graph()


def make_in_maps(
    query, key, value, mask,
    wq_kernel, wq_bias, wk_kernel, wk_bias,
    wv_kernel, wv_bias, wo_kernel, wo_bias,
):
    q = np.asarray(query, np.float32)
    k = np.asarray(key, np.float32)
    v = np.asarray(value, np.float32)
    mask = np.asarray(mask)
    wqk = np.asarray(wq_kernel, np.float32)
    wkk = np.asarray(wk_kernel, np.float32)
    wvk = np.asarray(wv_kernel, np.float32)
    wok = np.asarray(wo_kernel, np.float32)

    def tile_x(a):  # [S, D] -> [P, TDIN, S] pre-tiled transpose
        return np.ascontiguousarray(
            a.T.reshape(TDIN, P, S).transpose(1, 0, 2)
        ).astype(BF16)

    xt = [[tile_x(x[b]) for x in (q, k, v)] for b in range(B)]
    mt = [
        np.ascontiguousarray(mask[b].T.astype(np.float32)).astype(BF16)
        for b in range(B)
    ]
    in_maps = []
    for c in range(NCORES):
        b, g = divmod(c, GH)
        cs = slice(g * DG, (g + 1) * DG)
        # wo rows for this group: [256, D] -> [128, NT2, D] with partition
        # p = (h%2)*64+hd, tile t = h//2  (head pair packing).
        wog = wok[cs, :].reshape(HPG, HD, D)        # [h, hd, n]
        wo_arr = np.ascontiguousarray(
            wog.reshape(NT2, 2, HD, D)               # [t, h%2, hd, n]
            .transpose(1, 2, 0, 3)                   # [h%2, hd, t, n]
            .reshape(P, NT2, D)
        ).astype(BF16)
        # q/k biases as [128, NT2] per-partition columns (dout tiles)
        qb_arr = np.ascontiguousarray(
            np.asarray(wq_bias, np.float32)[cs].reshape(NT2, P).T
        )
        kb_arr = np.ascontiguousarray(
            np.asarray(wk_bias, np.float32)[cs].reshape(NT2, P).T
        )
        in_maps.append(
            {
                "xq_t": xt[b][0],
                "xk_t": xt[b][1],
                "xv_t": xt[b][2],
                "mask_t": mt[b],
                "wq": np.ascontiguousarray(wqk[:, cs].reshape(TDIN, P, DG).transpose(1, 0, 2)).astype(BF16),
                "wk": np.ascontiguousarray(wkk[:, cs].reshape(TDIN, P, DG).transpose(1, 0, 2)).astype(BF16),
                "wv": np.ascontiguousarray(wvk[:, cs].reshape(TDIN, P, DG).transpose(1, 0, 2)).astype(BF16),
                "wo": wo_arr,
                "qb": qb_arr,
                "kb": kb_arr,
                "vb": np.asarray(wv_bias, np.float32)[cs].reshape(1, DG).astype(BF16),
            }
        )
    return in_maps


def combine_outputs(results, wo_bias):
    outs = np.stack([np.asarray(r["out"], np.float32) for r in results])
    full = outs.reshape(B, GH, S, D).sum(axis=1)
    return (full + np.asarray(wo_bias, np.float32)[None, None, :]).astype(
        np.float32
    )


def kernel(**inputs):
    from concourse import bass_utils

    nc = _graph()
    in_maps = make_in_maps(**inputs)
    res = bass_utils.run_bass_kernel_spmd(
        nc, in_maps, core_ids=list(range(NCORES))
    )
    return combine_outputs(res.results, inputs["wo_bias"])


# revision 37
# speedup vs baseline: 1.0360x; 1.0360x over previous
# Multi-head attention (B=2, S=2048, D=1024, H=16) on 8 TRN2 NeuronCores.
#
# Sharding (hardcoded): core c in [0..8) handles batch b = c//4 and head
# group g = c%4 (4 heads = 256 output features of wq/wk/wv, 256 input rows
# of wo). Each core computes a partial output projection [S, D]; the host
# sums the 4 partials per batch and adds wo_bias (row-parallel unshard).
#
# Design (engines balanced around the Scalar exp floor, ~147us/core):
#   - all matmuls bf16 (fp8 fails the 2e-2 tolerance: random-sign dots keep
#     the per-element quant error, and scores enter exp).
#   - activations enter transposed ([D, S]); scores computed transposed
#     (S^T[k, q]) so softmax(P) feeds P@V directly; denominator comes free
#     as a ones-column appended to each head's V block.
#   - output projection contracts K=128 by packing head PAIRS into
#     otn2[128, t, S]; odd heads reach partitions 64..127 via a small
#     SBUF->SBUF DMA (engines cannot shift partitions).
#   - uniform deferred-by-one pipeline: each head's 16 score-kt "slots"
#     carry the PREVIOUS head's PV (2 kt-pairs/slot over slots 0-7) plus
#     V-proj tiles / K-t1 / Q-qc1 chunks / outproj tiles as PE fillers, so
#     exp sees a steady kt stream from ~12us in.
#   - softmax normalization split: reciprocal + PSUM-freeing copy + DRAM
#     bounce broadcast at slot 8; the consuming multiply at slot 15 so the
#     bounce latency never blocks the Vector stream head-of-line.
#   - last head: PV inlined (slots 8-15), reciprocal broadcast via a tiny
#     PE matmul (no DRAM bounce on the critical tail), outproj qc1 drains
#     on Scalar (exp is done by then).
import functools
import sys

import numpy as np

try:
    import concourse  # noqa: F401
except ImportError:  # harness env without the default path
    sys.path.insert(0, "/opt/trn_rl_repo")
    sys.path.insert(0, "/opt/pypackages")

import ml_dtypes

BF16 = ml_dtypes.bfloat16

B, S, D, H = 2, 2048, 1024, 16
HD = D // H          # 64
NCORES = 8
GH = 4               # head groups (tensor-parallel)
HPG = H // GH        # heads per group = 4
DG = D // GH         # features per group = 256
P = 128              # partitions
TDIN = D // P        # 8 din tiles
QC = 2               # q-chunks of 1024 for attention
QW = S // QC         # 1024
KT = S // P          # 16 k tiles
NT2 = DG // P        # 2 dout tiles per group


def build_graph():
    """Build the SPMD Bass graph (identical on all 8 cores)."""
    from contextlib import ExitStack

    from concourse import bacc, mybir, tile

    f32 = mybir.dt.float32
    bf16 = mybir.dt.bfloat16
    EXP = mybir.ActivationFunctionType.Exp
    IDENT = mybir.ActivationFunctionType.Identity

    nc = bacc.Bacc(
        "TRN2", target_bir_lowering=False, debug=False, num_devices=NCORES
    )

    xq = nc.dram_tensor("xq_t", (P, TDIN, S), bf16, kind="ExternalInput")
    xk = nc.dram_tensor("xk_t", (P, TDIN, S), bf16, kind="ExternalInput")
    xv = nc.dram_tensor("xv_t", (P, TDIN, S), bf16, kind="ExternalInput")
    mk = nc.dram_tensor("mask_t", (S, S), bf16, kind="ExternalInput")
    wq = nc.dram_tensor("wq", (P, TDIN, DG), bf16, kind="ExternalInput")
    wk = nc.dram_tensor("wk", (P, TDIN, DG), bf16, kind="ExternalInput")
    wv = nc.dram_tensor("wv", (P, TDIN, DG), bf16, kind="ExternalInput")
    # wo pre-arranged host-side to [128, NT2, D]: partition p = (h%2)*64+hd,
    # tile t = h//2 (head pair), so outproj contracts K=128 over 2 heads.
    wo = nc.dram_tensor("wo", (P, NT2, D), bf16, kind="ExternalInput")
    # q/k biases as per-partition columns [128, NT2]; v bias as a row.
    qb = nc.dram_tensor("qb", (P, NT2), f32, kind="ExternalInput")
    kb = nc.dram_tensor("kb", (P, NT2), f32, kind="ExternalInput")
    vb = nc.dram_tensor("vb", (1, DG), bf16, kind="ExternalInput")
    out = nc.dram_tensor("out", (S, D), bf16, kind="ExternalOutput")

    with tile.TileContext(nc) as tc, ExitStack() as ctx:
        wpool = ctx.enter_context(tc.tile_pool(name="wpool", bufs=1))
        cpool = ctx.enter_context(tc.tile_pool(name="cpool", bufs=1))
        qkpool = ctx.enter_context(tc.tile_pool(name="qk", bufs=1))
        vpool = ctx.enter_context(tc.tile_pool(name="vsb", bufs=1))
        mpool = ctx.enter_context(tc.tile_pool(name="msk", bufs=1))
        opool = ctx.enter_context(tc.tile_pool(name="otn", bufs=1))
        xstage = ctx.enter_context(tc.tile_pool(name="xin", bufs=2))
        ptpool = ctx.enter_context(tc.tile_pool(name="pt", bufs=16))
        npool = ctx.enter_context(tc.tile_pool(name="nrm", bufs=1))
        ospool = ctx.enter_context(tc.tile_pool(name="osbp", bufs=1))
        dpool = ctx.enter_context(tc.tile_pool(name="dscr", bufs=2, space="DRAM"))
        # PSUM: scores [128,1024] x2 (4 banks) + o_ps [65,1024] x1 (2 banks)
        # + proj/outproj [128,512] x2 (2 banks) = 8 banks exactly.
        spspool = ctx.enter_context(tc.tile_pool(name="sps", bufs=2, space="PSUM"))
        opspool = ctx.enter_context(tc.tile_pool(name="ops", bufs=1, space="PSUM"))
        pjpool = ctx.enter_context(tc.tile_pool(name="pjps", bufs=2, space="PSUM"))

        # ---- staging + persistent SBUF tensors ---------------------------
        def stage_x(xdr, c, tag, name, pieces=4):
            xt = xstage.tile([P, TDIN, 512], bf16, tag=tag, name=name)
            w = TDIN // pieces
            for th_ in range(pieces):
                nc.sync.dma_start(
                    xt[:, th_ * w : (th_ + 1) * w, :],
                    xdr.ap()[:, th_ * w : (th_ + 1) * w, c * 512 : (c + 1) * 512],
                )
            return xt

        xq_c = [stage_x(xq, 0, "xq", "xq_c0", pieces=8)]
        wq_sb = wpool.tile([P, TDIN, DG], bf16)
        wk_sb = wpool.tile([P, TDIN, DG], bf16)
        for th_ in range(4):
            nc.sync.dma_start(
                wq_sb[:, th_ * 2 : (th_ + 1) * 2, :],
                wq.ap()[:, th_ * 2 : (th_ + 1) * 2, :],
            )
        xq_c.append(stage_x(xq, 1, "xq", "xq_c1", pieces=8))
        for th_ in range(4):
            nc.sync.dma_start(
                wk_sb[:, th_ * 2 : (th_ + 1) * 2, :],
                wk.ap()[:, th_ * 2 : (th_ + 1) * 2, :],
            )
        xk_c = [stage_x(xk, 0, "xk", "xk_c0", pieces=8)]
        qb_sb = cpool.tile([P, NT2], f32)
        kb_sb = cpool.tile([P, NT2], f32)
        nc.sync.dma_start(qb_sb[:], qb.ap())
        nc.sync.dma_start(kb_sb[:], kb.ap())
        xk_c.append(stage_x(xk, 1, "xk", "xk_c1", pieces=8))

        # mask per-(kt, qc-half): only the qc0 halves are needed during
        # the first four heads, halving early DMA pressure.
        mask_sb = mpool.tile([P, KT, S], bf16)
        mk_r = mk.ap().rearrange("(t p) q -> p t q", p=P)

        def mask_dma(kt, qc):
            return lambda: nc.sync.dma_start(
                mask_sb[:, kt, qc * QW : (qc + 1) * QW],
                mk_r[:, kt, qc * QW : (qc + 1) * QW],
            )

        mask_dma(0, 0)()
        xv_pre = stage_x(xv, 0, "xv", "xv_c0")
        mask_dma(1, 0)()
        wv_sb = wpool.tile([P, TDIN, DG], bf16)
        for th_ in range(2):
            nc.sync.dma_start(
                wv_sb[:, th_ * 4 : (th_ + 1) * 4, :],
                wv.ap()[:, th_ * 4 : (th_ + 1) * 4, :],
            )
        wo_sb = wpool.tile([P, NT2, D], bf16)

        def wo_dma():
            return lambda: nc.sync.dma_start(wo_sb[:], wo.ap())

        vb_sb = cpool.tile([1, DG], bf16)
        nc.sync.dma_start(vb_sb[:], vb.ap())
        ones2 = cpool.tile([1, P], bf16)
        nc.vector.memset(ones2[:], 1.0)
        ones65 = cpool.tile([HD + 1, HD], f32)
        nc.vector.memset(ones65[:], 1.0)

        qT_sb = qkpool.tile([P, NT2, S], bf16)   # q projection, transposed
        kT_sb = qkpool.tile([P, NT2, S], bf16)
        # v blocks: per k-tile, per head: [v(64) | ones] -> 65 cols
        v_sb = vpool.tile([P, KT, HPG * (HD + 1)], bf16)
        nc.vector.memset(
            v_sb[:].rearrange("p s (h x) -> p s h x", h=HPG)[:, :, :, HD : HD + 1],
            1.0,
        )
        # packed normalized attention output: partition (h%2)*64+hd, tile h//2
        otn2 = opool.tile([P, NT2, S], bf16)

        # ---- projection helpers ------------------------------------------
        def emit_qk_chunk(xsb, xoff, wsb, bias_sb, dest, dt, s0, drain_eng):
            """Project one [128 dout, 512 s] tile: 8 acc matmuls + drain."""
            ps = pjpool.tile([P, 512], f32, tag="pj", name=f"pj_{dt}_{s0}_{drain_eng}")
            for ktl in range(TDIN):
                nc.tensor.matmul(
                    ps[:],
                    lhsT=wsb[:, ktl, dt * P : (dt + 1) * P],
                    rhs=xsb[:, ktl, s0 - xoff : s0 - xoff + 512],
                    start=(ktl == 0),
                    stop=(ktl == TDIN - 1),
                )
            if drain_eng == "scalar":
                nc.scalar.activation(
                    dest[:, dt, s0 : s0 + 512], ps[:], IDENT,
                    bias=bias_sb[:, dt : dt + 1], scale=1.0,
                )
            else:
                nc.vector.tensor_scalar_add(
                    dest[:, dt, s0 : s0 + 512], ps[:], bias_sb[:, dt : dt + 1]
                )

        def emit_v_mtile(xsb, xoff, st):
            """V projection for s-tile st ([128 s, 256 dout] + bias + ones)."""
            ps = pjpool.tile([P, 512], f32, tag="pj", name=f"pv_{st}")
            for ktl in range(TDIN):
                nc.tensor.matmul(
                    ps[:, 0:DG],
                    lhsT=xsb[:, ktl, st * P - xoff : (st + 1) * P - xoff],
                    rhs=wv_sb[:, ktl, :],
                    start=(ktl == 0),
                    stop=False,
                )
            nc.tensor.matmul(
                ps[:, 0:DG],
                lhsT=ones2[0:1, :],
                rhs=vb_sb[:],
                start=False,
                stop=True,
            )
            dst = v_sb[:, st, :].rearrange("p (h x) -> p h x", h=HPG)[:, :, 0:HD]
            src = ps[:, 0:DG].rearrange("p (h x) -> p h x", h=HPG)
            nc.vector.tensor_copy(dst, src)

        # ---- attention helpers -------------------------------------------
        def emit_scores_kt(qc, h, kt, gp_ok=True):
            """scores (2 matmuls) -> exp -> mask-mult; returns P tile."""
            t, po = h // 2, (h % 2) * HD
            s_ps = spspool.tile([P, QW], f32, tag="sps", name=f"sps_{qc}_{h}_{kt}")
            for hf in range(2):
                nc.tensor.matmul(
                    s_ps[:, hf * 512 : (hf + 1) * 512],
                    lhsT=kT_sb[po : po + HD, t, kt * P : (kt + 1) * P],
                    rhs=qT_sb[
                        po : po + HD, t,
                        qc * QW + hf * 512 : qc * QW + (hf + 1) * 512,
                    ],
                    start=True,
                    stop=True,
                )
            pt = ptpool.tile([P, QW], bf16, tag="p", name=f"pt_{qc}_{h}_{kt}")
            nc.scalar.activation(pt[:], s_ps[:], EXP, scale=0.125)
            meng = nc.gpsimd if (gp_ok and kt in (4, 9, 14)) else nc.vector
            meng.tensor_mul(
                pt[:], pt[:], mask_sb[:, kt, qc * QW : (qc + 1) * QW]
            )
            return pt

        def emit_pv_kt(h, kt, pt, o_ps):
            for hf in range(2):
                nc.tensor.matmul(
                    o_ps[:, hf * 512 : (hf + 1) * 512],
                    lhsT=v_sb[:, kt, h * 65 : (h + 1) * 65],
                    rhs=pt[:, hf * 512 : (hf + 1) * 512],
                    start=(kt == 0),
                    stop=(kt == KT - 1),
                )

        def emit_norm_a(qc, h, o_ps):
            """Reciprocal + PSUM-freeing copy + DRAM-bounce broadcast DMAs.
            The consuming multiply runs at slot 15, hiding bounce latency."""
            rec65 = npool.tile([HD + 1, QW], f32, tag="rec")
            nc.vector.reciprocal_approx_fast(out=rec65[:], in_=o_ps[:])
            osb = ospool.tile([HD + 1, QW], bf16, tag="osb", name=f"osb_{qc}_{h}")
            nc.vector.tensor_copy(osb[:], o_ps[:])
            scr = dpool.tile([1, QW], f32, tag="scr", name=f"scr_{qc}_{h}")
            nc.sync.dma_start(scr[:], rec65[HD : HD + 1, :])
            rb = npool.tile([HD, QW], f32, tag="rb")
            nc.sync.dma_start(rb[:], scr[:].to_broadcast((HD, QW)))
            return osb, rb

        def emit_norm_b(qc, h, osb, rb):
            """softmax normalization; writes otn2 (odd heads via DMA)."""
            t = h // 2
            if h % 2 == 0:
                nc.vector.tensor_mul(
                    otn2[0:HD, t, qc * QW : (qc + 1) * QW], osb[0:HD, :], rb[:]
                )
            else:
                nc.vector.tensor_mul(osb[0:HD, :], osb[0:HD, :], rb[:])
                nc.sync.dma_start(
                    otn2[HD:P, t, qc * QW : (qc + 1) * QW], osb[0:HD, :]
                )

        def emit_outproj(st, drain="vector"):
            osb2 = ospool.tile([P, D], bf16, tag="outsb", name=f"outsb_{st}",
                               bufs=2)
            for nch in range(2):
                op_ps = pjpool.tile(
                    [P, 512], f32, tag="pj", name=f"ops2_{st}_{nch}"
                )
                for t in range(NT2):
                    nc.tensor.matmul(
                        op_ps[:],
                        lhsT=otn2[:, t, st * P : (st + 1) * P],
                        rhs=wo_sb[:, t, nch * 512 : (nch + 1) * 512],
                        start=(t == 0),
                        stop=(t == NT2 - 1),
                    )
                dst = osb2[:, nch * 512 : (nch + 1) * 512]
                if drain == "scalar":
                    nc.scalar.copy(dst, op_ps[:])
                else:
                    nc.vector.tensor_copy(dst, op_ps[:])
            nc.sync.dma_start(out.ap()[st * P : (st + 1) * P, :], osb2[:])

        # ---- emission schedule -------------------------------------------
        seq = [(qc, h) for qc in range(QC) for h in range(HPG)]

        # prefix: all Q-qc0 + K-t0 chunk 0 only (~4MB of critical DMA);
        # K-t0 c1..c3 stream as n0 fillers just ahead of their score kts.
        for dt in range(NT2):
            for c in range(2):
                emit_qk_chunk(xq_c[c], c * 512, wq_sb, qb_sb, qT_sb, dt,
                              c * 512, "scalar")
        emit_qk_chunk(xk_c[0], 0, wk_sb, kb_sb, kT_sb, 0, 0, "scalar")

        def k0_chunk(c):
            return lambda: emit_qk_chunk(
                xk_c[c], c * 512, wk_sb, kb_sb, kT_sb, 0, c * 512, "vector")

        xv_c = [xv_pre]

        def v_tile(st):
            return lambda: emit_v_mtile(xv_c[st // 4], (st // 4) * 512, st)

        def stage_thunk(lst, xdr, c, tag, name):
            return lambda: lst.append(stage_x(xdr, c, tag, name))

        def k1_chunk(c):
            return lambda: emit_qk_chunk(
                xk_c[4 + c], c * 512, wk_sb, kb_sb, kT_sb, 1, c * 512,
                "vector")

        def q1_chunk(dt, c):
            return lambda: emit_qk_chunk(
                xq_c[c], c * 512, wq_sb, qb_sb, qT_sb, dt, c * 512,
                "vector")

        # filler map: fillers[n][slot] = list of thunks
        fillers = [dict() for _ in range(8)]

        def add(n, slot, thunk):
            fillers[n].setdefault(slot, []).append(thunk)

        # n0: K-t0 c1..c3 just ahead of their score kts; V tiles 0..7;
        # mask qc0-halves staggered; staging DMAs need-ordered.
        add(0, 0, stage_thunk(xk_c, xk, 2, "xk", "xk_c2"))
        add(0, 1, k0_chunk(1))
        add(0, 1, stage_thunk(xv_c, xv, 1, "xv", "xv_c1"))
        add(0, 4, stage_thunk(xk_c, xk, 3, "xk", "xk_c3"))
        add(0, 6, k0_chunk(2))
        add(0, 10, k0_chunk(3))
        for i, st in enumerate(range(8)):
            add(0, [2, 3, 5, 7, 9, 11, 12, 13][i], v_tile(st))
        for kt in range(2, KT):
            add(0, max(0, kt - 4), mask_dma(kt, 0))
        add(0, 8, stage_thunk(xv_c, xv, 2, "xv", "xv_c2"))
        add(0, 11, stage_thunk(xv_c, xv, 3, "xv", "xv_c3"))
        # n1: V tiles 8..15 (PV-n0-kt j runs at slot j, so V-kt by slot kt);
        # xk re-staging for K-t1; wo.
        for st in range(8, 16):
            add(1, st - 8, v_tile(st))
        add(1, 8, stage_thunk(xk_c, xk, 0, "xk", "xk_c0b"))
        add(1, 10, stage_thunk(xk_c, xk, 1, "xk", "xk_c1b"))
        add(1, 12, stage_thunk(xk_c, xk, 2, "xk", "xk_c2b"))
        add(1, 14, stage_thunk(xk_c, xk, 3, "xk", "xk_c3b"))
        add(1, 15, wo_dma())
        # n2: all K-t1 chunks; mask qc1-halves 0..7 staggered
        add(2, 0, k1_chunk(0))
        add(2, 3, k1_chunk(1))
        add(2, 6, k1_chunk(2))
        add(2, 9, k1_chunk(3))
        for kt in range(8):
            add(2, 2 * kt, mask_dma(kt, 1))
        # n3: all Q-qc1 chunks; mask qc1-halves 8..15
        add(3, 0, stage_thunk(xq_c, xq, 2, "xq", "xq_c2"))
        add(3, 2, stage_thunk(xq_c, xq, 3, "xq", "xq_c3"))
        add(3, 4, q1_chunk(0, 2))
        add(3, 7, q1_chunk(0, 3))
        add(3, 10, q1_chunk(1, 2))
        add(3, 13, q1_chunk(1, 3))
        for kt in range(8, KT):
            add(3, 2 * (kt - 8), mask_dma(kt, 1))
        # n5/n6: outproj qc0 tiles (norm-B of n3 lands at n5 slot 7)
        for i, st in enumerate(range(0, 4)):
            add(5, 9 + 2 * i, lambda st=st: emit_outproj(st))
        for i, st in enumerate(range(4, 8)):
            add(6, 9 + 2 * i, lambda st=st: emit_outproj(st))

        last = len(seq) - 1
        heads = []  # (qc, h, pts, o_ps) per head-seq index
        pnorm = None
        for n, (qc, h) in enumerate(seq):
            o_ps = opspool.tile([HD + 1, QW], f32, tag="ops",
                                name=f"ops_{qc}_{h}")
            pts = []
            for kt in range(KT):
                for thunk in fillers[n].get(kt, ()):
                    thunk()
                if n >= 1:
                    pqc, ph, ppts, po_ps = heads[n - 1]
                    if n == last:
                        if kt < 8:
                            for j in (2 * kt, 2 * kt + 1):
                                emit_pv_kt(ph, j, ppts[j], po_ps)
                    else:
                        emit_pv_kt(ph, kt, ppts[kt], po_ps)
                pts.append(emit_scores_kt(qc, h, kt, gp_ok=(n != last)))
                if n == last and kt == 8:
                    pqc, ph, ppts, po_ps = heads[n - 1]
                    pn6 = (pqc, ph) + emit_norm_a(pqc, ph, po_ps)
                if n == last and kt >= 8:
                    for j in (2 * (kt - 8), 2 * (kt - 8) + 1):
                        emit_pv_kt(h, j, pts[j], o_ps)
                if pnorm is not None and kt == 7:
                    emit_norm_b(*pnorm)
                    pnorm = None
                # norm-A of head n-1 right after its last PV pair: frees
                # the PSUM accumulator before the next window's PV starts.
                if n >= 1 and n != last and kt == 15:
                    pqc, ph, ppts, po_ps = heads[n - 1]
                    pnorm = (pqc, ph) + emit_norm_a(pqc, ph, po_ps)
                if n == last and kt == 15:
                    emit_norm_b(*pn6)
            heads.append((qc, h, pts, o_ps))
        prev = heads[-1]

        # tail: last head's norm via a PE broadcast of the reciprocal row
        # (no DRAM bounce), then outproj qc1 with Scalar drains.
        pqc, ph, ppts, po_ps = prev
        rec65 = npool.tile([HD + 1, QW], f32, tag="rec")
        nc.vector.reciprocal_approx_fast(out=rec65[:], in_=po_ps[:])
        osb = ospool.tile([HD + 1, QW], bf16, tag="osb", name="osb_last")
        nc.vector.tensor_copy(osb[:], po_ps[:])
        t = ph // 2
        for hf in range(2):
            rb_ps = pjpool.tile([HD, 512], f32, tag="pj", name=f"rbps_{hf}")
            nc.tensor.matmul(
                rb_ps[:],
                lhsT=ones65[HD : HD + 1, 0:HD],
                rhs=rec65[HD : HD + 1, hf * 512 : (hf + 1) * 512],
                start=True,
                stop=True,
            )
            nc.vector.tensor_mul(
                osb[0:HD, hf * 512 : (hf + 1) * 512],
                osb[0:HD, hf * 512 : (hf + 1) * 512],
                rb_ps[:],
            )
        nc.sync.dma_start(
            otn2[HD:P, t, pqc * QW : (pqc + 1) * QW], osb[0:HD, :]
        )
        for st in range(8, 16):
            emit_outproj(st, drain="scalar")

    nc.compile()
    return nc


@functools.lru_cache(maxsize=1)
def _graph():
    return build_graph()


def make_in_maps(
    query, key, value, mask,
    wq_kernel, wq_bias, wk_kernel, wk_bias,
    wv_kernel, wv_bias, wo_kernel, wo_bias,
):
    q = np.asarray(query, np.float32)
    k = np.asarray(key, np.float32)
    v = np.asarray(value, np.float32)
    mask = np.asarray(mask)
    wqk = np.asarray(wq_kernel, np.float32)
    wkk = np.asarray(wk_kernel, np.float32)
    wvk = np.asarray(wv_kernel, np.float32)
    wok = np.asarray(wo_kernel, np.float32)

    def tile_x(a):  # [S, D] -> [P, TDIN, S] pre-tiled transpose
        return np.ascontiguousarray(
            a.T.reshape(TDIN, P, S).transpose(1, 0, 2)
        ).astype(BF16)

    xt = [[tile_x(x[b]) for x in (q, k, v)] for b in range(B)]
    mt = [
        np.ascontiguousarray(mask[b].T.astype(np.float32)).astype(BF16)
        for b in range(B)
    ]
    in_maps = []
    for c in range(NCORES):
        b, g = divmod(c, GH)
        cs = slice(g * DG, (g + 1) * DG)
        # wo rows for this group: [256, D] -> [128, NT2, D] with partition
        # p = (h%2)*64+hd, tile t = h//2  (head pair packing).
        wog = wok[cs, :].reshape(HPG, HD, D)        # [h, hd, n]
        wo_arr = np.ascontiguousarray(
            wog.reshape(NT2, 2, HD, D)               # [t, h%2, hd, n]
            .transpose(1, 2, 0, 3)                   # [h%2, hd, t, n]
            .reshape(P, NT2, D)
        ).astype(BF16)
        # q/k biases as [128, NT2] per-partition columns (dout tiles)
        qb_arr = np.ascontiguousarray(
            np.asarray(wq_bias, np.float32)[cs].reshape(NT2, P).T
        )
        kb_arr = np.ascontiguousarray(
            np.asarray(wk_bias, np.float32)[cs].reshape(NT2, P).T
        )
        in_maps.append(
            {
                "xq_t": xt[b][0],
                "xk_t": xt[b][1],
                "xv_t": xt[b][2],
                "mask_t": mt[b],
                "wq": np.ascontiguousarray(wqk[:, cs].reshape(TDIN, P, DG).transpose(1, 0, 2)).astype(BF16),
                "wk": np.ascontiguousarray(wkk[:, cs].reshape(TDIN, P, DG).transpose(1, 0, 2)).astype(BF16),
                "wv": np.ascontiguousarray(wvk[:, cs].reshape(TDIN, P, DG).transpose(1, 0, 2)).astype(BF16),
                "wo": wo_arr,
                "qb": qb_arr,
                "kb": kb_arr,
                "vb": np.asarray(wv_bias, np.float32)[cs].reshape(1, DG).astype(BF16),
            }
        )
    return in_maps


def combine_outputs(results, wo_bias):
    outs = np.stack([np.asarray(r["out"], np.float32) for r in results])
    full = outs.reshape(B, GH, S, D).sum(axis=1)
    return (full + np.asarray(wo_bias, np.float32)[None, None, :]).astype(
        np.float32
    )


def kernel(**inputs):
    from concourse import bass_utils

    nc = _graph()
    in_maps = make_in_maps(**inputs)
    res = bass_utils.run_bass_kernel_spmd(
        nc, in_maps, core_ids=list(range(NCORES))
    )
    return combine_outputs(res.results, inputs["wo_bias"])
